# revision 5
# baseline (speedup 1.0000x reference)
"""AdaptiveGraphConv on 8 TRN2 NeuronCores (Bass/Tile) — v4.

Destination-sharded edge-parallel (as v3) with the mask stream eliminated:

- v3 streamed 58MB/core of host-built one-hot masks from HBM; together with
  the random gathers this saturated chip HBM (gathers ran at ~10ns/row vs
  4.4ns/row standalone). v4 builds both one-hots on chip:
    Gt [slot, dstc] = is_equal(iota2d, cc)   (one DVE tensor_scalar per tile)
    Gz [dstc, slot] = transpose(Gt)          (one TensorE transpose per tile)
  from a tiny cc stream ([P, NT] bf16, dst-slot per edge slot, 255 = pad).
- 2 region tables (25/24 groups, <32768 rows each for int16 gather idxs).
- Gather calls are per (chunk=2 groups, region) — ~2.3K idxs per call to
  amortize the ~0.5-1us fixed SWDGE call overhead (v3 used ~0.77K).
- wh is computed straight off the gather buffer (no early hcop copy).
- Everything else (phase-1 LN chain, progressive AllGathers, PSUM
  accumulation patterns, deferred gate/blend/LN tail) follows v3.
"""

import sys
import types

sys.path.insert(0, "/opt/trn_rl_repo")

import numpy as np
import ml_dtypes

import concourse.bass as bass
import concourse.bacc as bacc
import concourse.tile as tile
from concourse import mybir
from concourse.bass import AP
from concourse.bass_utils import run_bass_kernel_spmd

BF16 = ml_dtypes.bfloat16
F32 = mybir.dt.float32
BF = mybir.dt.bfloat16
I16 = mybir.dt.int16

N_CORES = 8
D = 128
P = 128
CHUNK_G = 2        # dst-groups per edge chunk
SB = 4             # tiles per z/relu batch
NREG = 2
REG_G = [25, 24]   # groups per region (NG=49)
PF = 2             # chunks prefetched ahead


def _install_ntff_hook():
    if "antenv.axon_hooks" in sys.modules:
        return
    try:
        from trn_agent_boot.trn_boot import _ntff_profile_via_ctypes

        hook = _ntff_profile_via_ctypes("/opt/axon/libaxon_pjrt.so")
    except Exception:
        hook = None
    mod = types.ModuleType("antenv.axon_hooks")
    mod.get_axon_ntff_profile_hook = lambda: hook
    mod.set_axon_ntff_profile_hook = lambda h: None
    sys.modules["antenv.axon_hooks"] = mod


def _bcast_d(ap):
    """append a stride-0 D dim to a [P, n] AP -> [P, n, D]"""
    return AP(ap.tensor, ap.offset, ap.ap + [[0, D]])


# ----------------------------------------------------------------------------
# device program
# ----------------------------------------------------------------------------

def _build_program(NG, NLOC, NPAD, TR, REG_B):
    # TR[r][g] tiles of group g gathered from region table r
    T_all = [sum(TR[r][g] for r in range(NREG)) for g in range(NG)]
    NT = sum(T_all)
    NTr = [sum(TR[r]) for r in range(NREG)]
    ROWS_R = [(REG_B[r + 1] - REG_B[r]) * P for r in range(NREG)]

    chunks = []
    g0 = 0
    while g0 < NG:
        chunks.append((g0, min(g0 + CHUNK_G, NG)))
        g0 = min(g0 + CHUNK_G, NG)
    TCr = [max(sum(TR[r][a:b]) for a, b in chunks) for r in range(NREG)]
    TC = sum(TCr)

    nc = bacc.Bacc(
        "TRN2", target_bir_lowering=False, debug=False, num_devices=N_CORES,
        num_swdge_queues=4,
    )

    def din(name, shape, dt):
        return nc.dram_tensor(name, list(shape), dt, kind="ExternalInput").ap()

    xT = din("xT", [P, NLOC], BF)
    W1 = din("W1", [P, D], BF)
    b1row = din("b1row", [1, D], BF)
    ones1b = din("ones1b", [1, D], BF)
    g1b = din("g1b", [P, D], F32)
    bt1b = din("bt1b", [P, D], F32)
    We1T = din("We1T", [P, D], BF)
    We1B = din("We1B", [P, D], BF)
    be1row = din("be1row", [1, D], BF)
    We2c = din("We2c", [P, 1], BF)
    be2c = din("be2c", [P, 1], F32)
    WgT = din("WgT", [P, D], BF)
    WgB = din("WgB", [P, D], BF)
    bgc = din("bgc", [P, 1], F32)
    g2b = din("g2b", [P, D], F32)
    bt2b = din("bt2b", [P, D], F32)
    idn = din("idn", [P, P], BF)
    iota2d = din("iota2d", [P, P], BF)
    ixr = [
        din(f"ix{r}", [P, max(NTr[r], 1) * 8], I16) for r in range(NREG)
    ]
    ccs = din("ccs", [P, NT], F32)

    out = nc.dram_tensor("out", [NLOC, D], F32, kind="ExternalOutput").ap()

    with tile.TileContext(nc, trace_sim=False) as tc:
        with (
            tc.tile_pool(name="singles", bufs=1) as sg,
            tc.tile_pool(name="dram", bufs=1, space="DRAM") as dram,
        ):
            def load(ap_in, shape, dt, name):
                t = sg.tile(list(shape), dt, name=name)
                nc.sync.dma_start(out=t[:], in_=ap_in[:])
                return t

            W1_sb = load(W1, [P, D], BF, "W1_sb")
            b1_sb = load(b1row, [1, D], BF, "b1_sb")
            o1b_sb = load(ones1b, [1, D], BF, "o1b_sb")
            g1_sb = load(g1b, [P, D], F32, "g1_sb")
            bt1_sb = load(bt1b, [P, D], F32, "bt1_sb")
            We1T_sb = load(We1T, [P, D], BF, "We1T_sb")
            We1B_sb = load(We1B, [P, D], BF, "We1B_sb")
            be1_sb = load(be1row, [1, D], BF, "be1_sb")
            We2_sb = load(We2c, [P, 1], BF, "We2_sb")
            be2_sb = load(be2c, [P, 1], F32, "be2_sb")
            WgT_sb = load(WgT, [P, D], BF, "WgT_sb")
            WgB_sb = load(WgB, [P, D], BF, "WgB_sb")
            bg_sb = load(bgc, [P, 1], F32, "bg_sb")
            g2_sb = load(g2b, [P, D], F32, "g2_sb")
            bt2_sb = load(bt2b, [P, D], F32, "bt2_sb")
            idn_sb = load(idn, [P, P], BF, "idn_sb")
            iota_sb = load(iota2d, [P, P], BF, "iota_sb")
            ix_sb = [
                load(ixr[r], [P, max(NTr[r], 1) * 8], I16, f"ix{r}_sb")
                for r in range(NREG)
            ]
            cc_sb = load(ccs, [P, NT], F32, "cc_sb")

            eps_sb = sg.tile([P, 1], F32, name="eps_sb")
            nc.vector.memset(eps_sb[:], 1e-5)

            hT_sb = sg.tile([P, NLOC], BF, name="hT_sb")
            B_sb = sg.tile([P, NLOC], BF, name="B_sb")
            RW = max(REG_B[r + 1] - REG_B[r] for r in range(NREG)) * P
            hreg_sb = sg.tile([P, RW], BF, name="hreg_sb")
            areg_sb = sg.tile([P, RW], BF, name="areg_sb")

            HA_shard = dram.tile([NLOC, 2 * D], BF, name="HA_shard")
            HA_tab = [
                dram.tile(
                    [N_CORES * ROWS_R[r], 2 * D], BF, name=f"HA_t{r}",
                    addr_space="Shared",
                )
                for r in range(NREG)
            ]

            # ======== phase 1 + edges (one scope; gathers interleave) ========
            with (
                tc.tile_pool(name="pz", bufs=2, space="PSUM") as pz,
                tc.tile_pool(name="psc", bufs=1, space="PSUM") as psc,
                tc.tile_pool(name="pag", bufs=1, space="PSUM") as pag,
                tc.tile_pool(name="pg3", bufs=1, space="PSUM") as pg3,
                tc.tile_pool(name="pgz", bufs=2, space="PSUM") as pgz,
                tc.tile_pool(name="gio", bufs=PF + 1) as gio,
                tc.tile_pool(name="gmk", bufs=PF + 1) as gmk,
                tc.tile_pool(name="wrk", bufs=3) as wrk,
                tc.tile_pool(name="osb", bufs=2) as osb,
            ):
                qctr = [0]

                def next_q():
                    q = qctr[0] % 4
                    qctr[0] += 1
                    return q

                # per-chunk buffer bookkeeping
                REG_OFF = [[0] for _ in range(NREG)]
                TOFF = [0]
                for (a, b) in chunks:
                    for r in range(NREG):
                        REG_OFF[r].append(REG_OFF[r][-1] + sum(TR[r][a:b]))
                    TOFF.append(TOFF[-1] + sum(T_all[a:b]))
                cbufs = {}

                def alloc_chunk(ci):
                    hab = [
                        gio.tile(
                            [P, max(TCr[r], 1), 2 * D], BF, tag=f"ha{r}",
                            name=f"ha{r}_{ci}",
                        )
                        for r in range(NREG)
                    ]
                    gtb = gmk.tile(
                        [P, max(TC, 1), P], BF, tag="gt", name=f"gt{ci}",
                    )
                    cbufs[ci] = (hab, gtb)
                    return hab, gtb

                def issue_gather(ci, r):
                    n = REG_OFF[r][ci + 1] - REG_OFF[r][ci]
                    if n:
                        off = REG_OFF[r][ci]
                        nc.gpsimd.dma_gather(
                            out_ap=cbufs[ci][0][r][:, 0:n, :],
                            in_ap=HA_tab[r][:, :],
                            idxs_ap=ix_sb[r][:, off * 8 : (off + n) * 8],
                            num_idxs=n * P,
                            num_idxs_reg=n * P,
                            elem_size=2 * D,
                            single_packet=False,
                            queue_num=next_q(),
                        )

                with tc.tile_pool(name="w1p", bufs=3) as w1p:
                  ha_v = HA_shard.rearrange("(g p) c -> p g c", p=P)
                  for r4 in range(NREG):
                      for g in range(REG_B[r4], REG_B[r4 + 1]):
                          gsl = slice(g * P, (g + 1) * P)
                          xg = w1p.tile([P, D], BF, tag="xg", name=f"xg{g}")
                          nc.sync.dma_start(out=xg[:], in_=xT[:, gsl])
                          hp = pz.tile([P, D], F32, tag="z", name=f"hp{g}")
                          nc.tensor.matmul(
                              out=hp[:], lhsT=xg[:], rhs=W1_sb[:],
                              start=True, stop=False,
                          )
                          nc.tensor.matmul(
                              out=hp[:], lhsT=o1b_sb[:], rhs=b1_sb[:],
                              start=False, stop=True,
                          )
                          st = w1p.tile([P, 6], F32, tag="st", name=f"st{g}")
                          nc.vector.bn_stats(out=st[:], in_=hp[:])
                          mv = w1p.tile([P, 2], F32, tag="mv", name=f"mv{g}")
                          nc.vector.bn_aggr(out=mv[:], in_=st[:])
                          sd = w1p.tile([P, 1], F32, tag="sd", name=f"sd{g}")
                          nc.scalar.activation(
                              out=sd[:], in_=mv[:, 1:2],
                              func=mybir.ActivationFunctionType.Sqrt,
                              bias=eps_sb[:],
                          )
                          rstd = w1p.tile([P, 1], F32, tag="rstd", name=f"rs{g}")
                          nc.vector.reciprocal(out=rstd[:], in_=sd[:])
                          t1 = w1p.tile([P, D], F32, tag="t1", name=f"t1{g}")
                          nc.vector.tensor_scalar(
                              out=t1[:], in0=hp[:], scalar1=mv[:, 0:1],
                              scalar2=rstd[:], op0=mybir.AluOpType.subtract,
                              op1=mybir.AluOpType.mult,
                          )
                          u1 = w1p.tile([P, D], F32, tag="u1", name=f"u1{g}")
                          nc.vector.tensor_mul(out=u1[:], in0=t1[:], in1=g1_sb[:])
                          v1 = w1p.tile([P, D], F32, tag="v1", name=f"v1{g}")
                          nc.vector.tensor_add(out=v1[:], in0=u1[:], in1=bt1_sb[:])
                          gl = slice((g - REG_B[r4]) * P, (g - REG_B[r4] + 1) * P)
                          hgt = hreg_sb[:, gl]
                          nc.vector.tensor_scalar_max(
                              out=hgt, in0=v1[:], scalar1=0.0
                          )
                          htp = pag.tile([P, D], F32, tag="aggr", name=f"htp{g}")
                          nc.tensor.matmul(
                              out=htp[:], lhsT=hgt, rhs=idn_sb[:],
                              start=True, stop=True,
                          )
                          nc.any.tensor_copy(out=hT_sb[:, gsl], in_=htp[:])
                          ap_ = psc.tile([P, D], F32, tag="s", name=f"apz{g}")
                          nc.tensor.matmul(
                              out=ap_[:], lhsT=hT_sb[:, gsl], rhs=We1T_sb[:],
                              start=True, stop=False,
                          )
                          nc.tensor.matmul(
                              out=ap_[:], lhsT=o1b_sb[:], rhs=be1_sb[:],
                              start=False, stop=True,
                          )
                          nc.any.tensor_copy(out=areg_sb[:, gl], in_=ap_[:])
                          bp = psc.tile([P, D], F32, tag="s", name=f"bp{g}")
                          nc.tensor.matmul(
                              out=bp[:], lhsT=hT_sb[:, gsl], rhs=We1B_sb[:],
                              start=True, stop=True,
                          )
                          nc.any.tensor_copy(out=B_sb[:, gsl], in_=bp[:])
                      # stage the region, then all-gather it
                      ga4, gb4 = REG_B[r4], REG_B[r4 + 1]
                      r0, r1 = ga4 * P, gb4 * P
                      ng4 = gb4 - ga4
                      nc.sync.dma_start(
                          out=ha_v[:, ga4:gb4, 0:D],
                          in_=hreg_sb[:, 0 : ng4 * P].rearrange(
                              "p (g j) -> p g j", g=ng4
                          ),
                      )
                      nc.sync.dma_start(
                          out=ha_v[:, ga4:gb4, D : 2 * D],
                          in_=areg_sb[:, 0 : ng4 * P].rearrange(
                              "p (g j) -> p g j", g=ng4
                          ),
                      )
                      nc.gpsimd.collective_compute(
                          "AllGather",
                          mybir.AluOpType.bypass,
                          replica_groups=[list(range(N_CORES))],
                          ins=[HA_shard[r0:r1, :].opt()],
                          outs=[HA_tab[r4][:, :].opt()],
                      )
                      # stagger pre-issued gathers so the next collective's
                      # dispatch is never head-blocked on the GpSimd queue
                      if r4 == 0:
                          for ci in range(min(PF, len(chunks))):
                              alloc_chunk(ci)
                      for ci in range(min(PF, len(chunks))):
                          issue_gather(ci, r4)

                def emit_tail(g, raggt):
                    gsl = slice(g * P, (g + 1) * P)
                    gp = pg3.tile([P, P], F32, tag="gp", name=f"gp{g}")
                    nc.tensor.matmul(
                        out=gp[:], lhsT=WgT_sb[:], rhs=hT_sb[:, gsl],
                        start=True, stop=False,
                    )
                    nc.tensor.matmul(
                        out=gp[:], lhsT=WgB_sb[:], rhs=raggt[:],
                        start=False, stop=True,
                    )
                    gate = wrk.tile([P, P], BF, tag="gate", name=f"gt_{g}")
                    nc.scalar.activation(
                        out=gate[:], in_=gp[:],
                        func=mybir.ActivationFunctionType.Sigmoid,
                        bias=bg_sb[:],
                    )
                    d1 = wrk.tile([P, P], BF, tag="d1", name=f"d1{g}")
                    nc.vector.tensor_tensor(
                        out=d1[:], in0=raggt[:], in1=hT_sb[:, gsl],
                        op=mybir.AluOpType.subtract,
                    )
                    d2 = wrk.tile([P, P], BF, tag="d2", name=f"d2{g}")
                    nc.vector.tensor_mul(out=d2[:], in0=gate[:], in1=d1[:])
                    hn = wrk.tile([P, P], BF, tag="hn", name=f"hn{g}")
                    nc.vector.tensor_add(
                        out=hn[:], in0=hT_sb[:, gsl], in1=d2[:]
                    )
                    hnp = pg3.tile([P, P], F32, tag="hnp", name=f"hnp{g}")
                    nc.tensor.matmul(
                        out=hnp[:], lhsT=hn[:], rhs=idn_sb[:],
                        start=True, stop=True,
                    )
                    st3 = wrk.tile([P, 6], F32, tag="st3", name=f"st3{g}")
                    nc.vector.bn_stats(out=st3[:], in_=hnp[:])
                    mv3 = wrk.tile([P, 2], F32, tag="mv3", name=f"mv3{g}")
                    nc.vector.bn_aggr(out=mv3[:], in_=st3[:])
                    sd3 = wrk.tile([P, 1], F32, tag="sd3", name=f"sd3{g}")
                    nc.scalar.activation(
                        out=sd3[:], in_=mv3[:, 1:2],
                        func=mybir.ActivationFunctionType.Sqrt,
                        bias=eps_sb[:],
                    )
                    rstd3 = wrk.tile([P, 1], F32, tag="rst3", name=f"rt{g}")
                    nc.vector.reciprocal(out=rstd3[:], in_=sd3[:])
                    t1o = osb.tile([P, D], F32, tag="t1o", name=f"t1o{g}")
                    nc.vector.tensor_scalar(
                        out=t1o[:], in0=hnp[:],
                        scalar1=mv3[:, 0:1], scalar2=rstd3[:],
                        op0=mybir.AluOpType.subtract,
                        op1=mybir.AluOpType.mult,
                    )
                    u1o = osb.tile([P, D], F32, tag="u1o", name=f"u1o{g}")
                    nc.vector.tensor_mul(out=u1o[:], in0=t1o[:], in1=g2_sb[:])
                    o1o = osb.tile([P, D], F32, tag="o1o", name=f"o1o{g}")
                    nc.vector.tensor_add(out=o1o[:], in0=u1o[:], in1=bt2_sb[:])
                    nc.sync.dma_start(out=out[gsl, :], in_=o1o[:])

                prev = None
                for ci, (ga, gb) in enumerate(chunks):
                    if ci + PF < len(chunks):
                        alloc_chunk(ci + PF)
                        for r in range(NREG):
                            issue_gather(ci + PF, r)
                    habuf, gtbuf = cbufs.pop(ci)
                    # build Gt for the whole chunk up front (DVE only)
                    ntc = TOFF[ci + 1] - TOFF[ci]
                    for tt in range(ntc):
                        nc.vector.tensor_scalar(
                            out=gtbuf[:, tt, :], in0=iota_sb[:],
                            scalar1=cc_sb[:, TOFF[ci] + tt : TOFF[ci] + tt + 1],
                            scalar2=None,
                            op0=mybir.AluOpType.is_equal,
                        )
                    for g in range(ga, gb):
                        tr = [TR[r][g] for r in range(NREG)]
                        Tg = sum(tr)
                        gsl = slice(g * P, (g + 1) * P)
                        _rp = [
                            sum(TR[r][ga:g]) for r in range(NREG)
                        ]
                        _m = sum(T_all[ga:g])
                        c0 = tr[0]

                        def hat(j, _rp=_rp, _c0=c0):
                            if j < _c0:
                                return habuf[0][:, _rp[0] + j, :]
                            return habuf[1][:, _rp[1] + j - _c0, :]

                        s_ps = psc.tile([P, Tg], F32, tag="s", name=f"s{g}")
                        w_sb = wrk.tile([P, Tg], BF, tag="w", name=f"w{g}")
                        nck = (Tg + SB - 1) // SB
                        for c in range(nck):
                            tl = c * SB
                            th = min(tl + SB, Tg)
                            wl = (th - tl) * P
                            # Gz for this batch: transpose Gt tiles on TensorE
                            gz = wrk.tile(
                                [P, SB, P], BF, tag="gz", name=f"gz{g}_{c}"
                            )
                            for i, t in enumerate(range(tl, th)):
                                gzp = pgz.tile(
                                    [P, P], BF, tag="gzp", name=f"gzp{g}_{t}"
                                )
                                nc.tensor.transpose(
                                    out=gzp[:], in_=gtbuf[:, _m + t, :],
                                    identity=idn_sb[:],
                                )
                                nc.any.tensor_copy(
                                    out=gz[:, i, :], in_=gzp[:]
                                )
                            z = pz.tile(
                                [P, SB * P], F32, tag="z", name=f"z{g}_{c}"
                            )
                            # start=True resets the whole PSUM bank's
                            # has_written bits: full-region write first,
                            # per-slice accumulates after
                            nc.tensor.matmul(
                                out=z[:, 0:wl], lhsT=B_sb[:, gsl],
                                rhs=gz[:, 0 : th - tl, :],
                                start=True, stop=False,
                            )
                            for i, t in enumerate(range(tl, th)):
                                zsl = slice(i * P, (i + 1) * P)
                                nc.tensor.matmul(
                                    out=z[:, zsl], lhsT=hat(t)[:, D : 2 * D],
                                    rhs=idn_sb[:], start=False,
                                    stop=(t == th - 1),
                                )
                            r_ = wrk.tile(
                                [P, SB * P], BF, tag="r", name=f"r{g}_{c}"
                            )
                            nc.vector.tensor_scalar_max(
                                out=r_[:, 0:wl], in0=z[:, 0:wl], scalar1=0.0
                            )
                            for i, t in enumerate(range(tl, th)):
                                nc.tensor.matmul(
                                    out=s_ps[:, t : t + 1],
                                    lhsT=r_[:, i * P : (i + 1) * P],
                                    rhs=We2_sb[:], start=True, stop=True,
                                )
                        nc.scalar.activation(
                            out=w_sb[:], in_=s_ps[:, 0:Tg],
                            func=mybir.ActivationFunctionType.Sigmoid,
                            bias=be2_sb[:],
                        )
                        # deferred tail of the previous group first, so
                        # its Vector ops aren't stuck behind wh's sem wait
                        if prev is not None:
                            emit_tail(*prev)
                        # weighted source tiles: broadcast tensor_tensor
                        # multiply (w stride-0 along D) off the gather bufs
                        wh = wrk.tile([P, Tg, D], BF, tag="hc", name=f"wh{g}")
                        woff0 = 0
                        for r in range(NREG):
                            if tr[r]:
                                nc.vector.tensor_tensor(
                                    out=wh[:, woff0 : woff0 + tr[r], :],
                                    in0=habuf[r][
                                        :, _rp[r] : _rp[r] + tr[r], 0:D
                                    ],
                                    in1=_bcast_d(w_sb[:, woff0 : woff0 + tr[r]]),
                                    op=mybir.AluOpType.mult,
                                )
                                woff0 += tr[r]
                        aggr = pag.tile([P, P], F32, tag="aggr", name=f"ag{g}")
                        for t in range(Tg):
                            nc.tensor.matmul(
                                out=aggr[:], lhsT=wh[:, t, :],
                                rhs=gtbuf[:, _m + t, :],
                                start=(t == 0), stop=(t == Tg - 1),
                            )
                        raggt = wrk.tile([P, P], BF, tag="ragg", name=f"rg{g}")
                        nc.vector.tensor_copy(out=raggt[:], in_=aggr[:])
                        prev = (g, raggt)
                emit_tail(*prev)

    nc.compile()
    return nc


# ----------------------------------------------------------------------------
# host-side sharding + launch
# ----------------------------------------------------------------------------

_CACHE = {}


def _wrap16(seq):
    """idx i -> [i%16, i//16], replicated to 128 partitions (8 Q7 cores)."""
    n = len(seq)
    if n == 0:
        return np.zeros((P, 8), np.int16)
    assert n % 16 == 0
    blk = np.asarray(seq, np.int16).reshape(-1, 16).T
    return np.tile(blk, (8, 1))


def kernel(
    x, edge_index, W1, b1, g1, bt1, We1, be1, We2, be2,
    Wn1, bn1, Wn2, bn2, Wg, bg, g2, bt2, _trace=False,
):
    x = np.asarray(x, dtype=np.float32)
    N = x.shape[0]
    NG = (N + N_CORES * P - 1) // (N_CORES * P)
    NLOC = NG * P
    NPAD = NLOC * N_CORES

    assert sum(REG_G) == NG
    REG_B = [0]
    for s in REG_G:
        REG_B.append(REG_B[-1] + s)
    ROWS_R = [REG_G[r] * P for r in range(NREG)]     # rows per core per region
    assert all(N_CORES * rr <= 32768 for rr in ROWS_R)

    row = np.asarray(edge_index[0], dtype=np.int64)
    col = np.asarray(edge_index[1], dtype=np.int64)
    order = np.argsort(col, kind="stable")
    row_s = row[order].astype(np.int64)
    col_s = col[order].astype(np.int64)
    bounds = np.searchsorted(col_s, np.arange(N_CORES + 1) * NLOC)

    # remap src node -> (region table, row within table)
    k_s = row_s // NLOC
    r_s = row_s % NLOC
    gr_s = r_s // P
    reg_s = np.searchsorted(np.asarray(REG_B[1:]), gr_s, side="right")
    rows_r_a = np.asarray(ROWS_R)
    reg_b_a = np.asarray(REG_B)
    tabrow = (k_s * rows_r_a[reg_s] + (r_s - reg_b_a[reg_s] * P)).astype(
        np.int64
    )

    # per-core per-group per-region counts -> shared tile counts
    cnt = np.zeros((NREG, N_CORES, NG), np.int64)
    for k in range(N_CORES):
        lo, hi = bounds[k], bounds[k + 1]
        gk = (col_s[lo:hi] - k * NLOC) // P
        rg = reg_s[lo:hi]
        for r in range(NREG):
            cnt[r, k] = np.bincount(gk[rg == r], minlength=NG)
    TR = [
        [int(v) for v in np.ceil(cnt[r].max(axis=0) / P)] for r in range(NREG)
    ]
    for g in range(NG):
        if sum(TR[r][g] for r in range(NREG)) == 0:
            TR[0][g] = 1
    T_all = [sum(TR[r][g] for r in range(NREG)) for g in range(NG)]
    NT = sum(T_all)
    toff = np.concatenate([[0], np.cumsum(T_all)]).astype(np.int64)

    key = (N, NG, tuple(tuple(TR[r]) for r in range(NREG)))
    if key not in _CACHE:
        _CACHE[key] = _build_program(NG, NLOC, NPAD, TR, REG_B)
    nc = _CACHE[key]

    bf = lambda a: np.ascontiguousarray(np.asarray(a, np.float32)).astype(BF16)
    f32 = lambda a: np.ascontiguousarray(np.asarray(a, np.float32))
    shared = {
        "W1": bf(W1),
        "b1row": bf(b1).reshape(1, D),
        "ones1b": np.ones((1, D), BF16),
        "g1b": np.broadcast_to(f32(g1).reshape(1, D), (P, D)).copy(),
        "bt1b": np.broadcast_to(f32(bt1).reshape(1, D), (P, D)).copy(),
        "We1T": bf(We1[:D]),
        "We1B": bf(We1[D:]),
        "be1row": bf(be1).reshape(1, D),
        "We2c": bf(We2).reshape(P, 1),
        "be2c": np.broadcast_to(f32(be2).reshape(1, 1), (P, 1)).copy(),
        "WgT": bf(Wg[:D]),
        "WgB": bf(Wg[D:]),
        "bgc": f32(bg).reshape(P, 1),
        "g2b": np.broadcast_to(f32(g2).reshape(1, D), (P, D)).copy(),
        "bt2b": np.broadcast_to(f32(bt2).reshape(1, D), (P, D)).copy(),
        "idn": np.eye(P, dtype=BF16),
        "iota2d": np.broadcast_to(
            np.arange(P, dtype=np.float32).reshape(1, P), (P, P)
        ).astype(BF16).copy(),
    }

    xp = np.zeros((NPAD, D), np.float32)
    xp[:N] = x

    in_maps = []
    for k in range(N_CORES):
        lo, hi = bounds[k], bounds[k + 1]
        rk = tabrow[lo:hi]
        regk = reg_s[lo:hi]
        ck = col_s[lo:hi] - k * NLOC
        gk = ck // P
        crel = (ck % P).astype(np.int64)
        seqs = [[] for _ in range(NREG)]
        ccv = np.full((NT, P), 255.0, np.float32)
        for g in range(NG):
            gm = gk == g
            r_g = rk[gm]
            c_g = crel[gm]
            reg_g = regk[gm]
            woff = 0
            for r in range(NREG):
                sel = reg_g == r
                Tcnt = TR[r][g]
                rr = r_g[sel]
                cc = c_g[sel]
                o = np.argsort(rr, kind="stable")
                rr = rr[o]
                cc = cc[o]
                n = len(rr)
                idxs = np.zeros(Tcnt * P, np.int64)
                idxs[:n] = rr
                seqs[r].append(idxs)
                if n:
                    e = np.arange(n)
                    t_proc = toff[g] + woff + e // P
                    slot = e % P
                    ccv[t_proc, slot] = cc
                woff += Tcnt
        im = dict(shared)
        im["xT"] = np.ascontiguousarray(
            xp[k * NLOC : (k + 1) * NLOC].T
        ).astype(BF16)
        for r in range(NREG):
            s = (
                np.concatenate(seqs[r]) if seqs[r] else np.zeros(0, np.int64)
            )
            im[f"ix{r}"] = _wrap16(s)
        im["ccs"] = np.ascontiguousarray(ccv.T)
        in_maps.append(im)

    if _trace:
        _install_ntff_hook()
    res = run_bass_kernel_spmd(
        nc, in_maps, core_ids=list(range(N_CORES)), trace=_trace
    )
    out = np.concatenate(
        [res.results[k]["out"] for k in range(N_CORES)], axis=0
    )[:N]
    if _trace:
        kernel.last_exec_time_ns = res.exec_time_ns
    return np.ascontiguousarray(out, dtype=np.float32)


# revision 6
# speedup vs baseline: 1.3382x; 1.3382x over previous
"""AdaptiveGraphConv on 8 TRN2 NeuronCores (Bass/Tile) — v4.

Destination-sharded edge-parallel (as v3) with the mask stream eliminated:

- v3 streamed 58MB/core of host-built one-hot masks from HBM; together with
  the random gathers this saturated chip HBM (gathers ran at ~10ns/row vs
  4.4ns/row standalone). v4 builds both one-hots on chip:
    Gt [slot, dstc] = is_equal(iota2d, cc)   (one DVE tensor_scalar per tile)
    Gz [dstc, slot] = transpose(Gt)          (one TensorE transpose per tile)
  from a tiny cc stream ([P, NT] bf16, dst-slot per edge slot, 255 = pad).
- 2 region tables (25/24 groups, <32768 rows each for int16 gather idxs).
- Gather calls are per (chunk=2 groups, region) — ~2.3K idxs per call to
  amortize the ~0.5-1us fixed SWDGE call overhead (v3 used ~0.77K).
- wh is computed straight off the gather buffer (no early hcop copy).
- Everything else (phase-1 LN chain, progressive AllGathers, PSUM
  accumulation patterns, deferred gate/blend/LN tail) follows v3.
"""

import sys
import types

sys.path.insert(0, "/opt/trn_rl_repo")

import numpy as np
import ml_dtypes

import concourse.bass as bass
import concourse.bacc as bacc
import concourse.tile as tile
from concourse import mybir
from concourse.bass import AP
from concourse.bass_utils import run_bass_kernel_spmd

BF16 = ml_dtypes.bfloat16
F32 = mybir.dt.float32
BF = mybir.dt.bfloat16
I16 = mybir.dt.int16

N_CORES = 8
D = 128
P = 128
CHUNK_G = 2        # dst-groups per edge chunk
SB = 4             # tiles per z/relu batch
NREG = 2
REG_G = [25, 24]   # groups per region (NG=49)
PF = 2             # chunks prefetched ahead


def _install_ntff_hook():
    if "antenv.axon_hooks" in sys.modules:
        return
    try:
        from trn_agent_boot.trn_boot import _ntff_profile_via_ctypes

        hook = _ntff_profile_via_ctypes("/opt/axon/libaxon_pjrt.so")
    except Exception:
        hook = None
    mod = types.ModuleType("antenv.axon_hooks")
    mod.get_axon_ntff_profile_hook = lambda: hook
    mod.set_axon_ntff_profile_hook = lambda h: None
    sys.modules["antenv.axon_hooks"] = mod


def _bcast_d(ap):
    """append a stride-0 D dim to a [P, n] AP -> [P, n, D]"""
    return AP(ap.tensor, ap.offset, ap.ap + [[0, D]])


# ----------------------------------------------------------------------------
# device program
# ----------------------------------------------------------------------------

def _build_program(NG, NLOC, NPAD, TR, REG_B):
    # TR[r][g] tiles of group g gathered from region table r
    T_all = [sum(TR[r][g] for r in range(NREG)) for g in range(NG)]
    NT = sum(T_all)
    NTr = [sum(TR[r]) for r in range(NREG)]
    ROWS_R = [(REG_B[r + 1] - REG_B[r]) * P for r in range(NREG)]

    chunks = []
    g0 = 0
    while g0 < NG:
        chunks.append((g0, min(g0 + CHUNK_G, NG)))
        g0 = min(g0 + CHUNK_G, NG)
    TCr = [max(sum(TR[r][a:b]) for a, b in chunks) for r in range(NREG)]
    TC = sum(TCr)

    nc = bacc.Bacc(
        "TRN2", target_bir_lowering=False, debug=False, num_devices=N_CORES,
        num_swdge_queues=4,
    )

    def din(name, shape, dt):
        return nc.dram_tensor(name, list(shape), dt, kind="ExternalInput").ap()

    xT = din("xT", [P, NLOC], BF)
    W1 = din("W1", [P, D], BF)
    b1row = din("b1row", [1, D], BF)
    ones1b = din("ones1b", [1, D], BF)
    g1b = din("g1b", [P, D], F32)
    bt1b = din("bt1b", [P, D], F32)
    We1T = din("We1T", [P, D], BF)
    We1B = din("We1B", [P, D], BF)
    be1row = din("be1row", [1, D], BF)
    We2c = din("We2c", [P, 1], BF)
    be2c = din("be2c", [P, 1], F32)
    WgT = din("WgT", [P, D], BF)
    WgB = din("WgB", [P, D], BF)
    bgc = din("bgc", [P, 1], F32)
    g2b = din("g2b", [P, D], F32)
    bt2b = din("bt2b", [P, D], F32)
    idn = din("idn", [P, P], BF)
    ixr = [
        din(f"ix{r}", [P, max(NTr[r], 1) * 8], I16) for r in range(NREG)
    ]
    masks = din("masks", [P, NT * 2 * P], BF)

    out = nc.dram_tensor("out", [NLOC, D], F32, kind="ExternalOutput").ap()

    with tile.TileContext(nc, trace_sim=False) as tc:
        with (
            tc.tile_pool(name="singles", bufs=1) as sg,
            tc.tile_pool(name="dram", bufs=1, space="DRAM") as dram,
        ):
            def load(ap_in, shape, dt, name):
                t = sg.tile(list(shape), dt, name=name)
                nc.sync.dma_start(out=t[:], in_=ap_in[:])
                return t

            W1_sb = load(W1, [P, D], BF, "W1_sb")
            b1_sb = load(b1row, [1, D], BF, "b1_sb")
            o1b_sb = load(ones1b, [1, D], BF, "o1b_sb")
            g1_sb = load(g1b, [P, D], F32, "g1_sb")
            bt1_sb = load(bt1b, [P, D], F32, "bt1_sb")
            We1T_sb = load(We1T, [P, D], BF, "We1T_sb")
            We1B_sb = load(We1B, [P, D], BF, "We1B_sb")
            be1_sb = load(be1row, [1, D], BF, "be1_sb")
            We2_sb = load(We2c, [P, 1], BF, "We2_sb")
            be2_sb = load(be2c, [P, 1], F32, "be2_sb")
            WgT_sb = load(WgT, [P, D], BF, "WgT_sb")
            WgB_sb = load(WgB, [P, D], BF, "WgB_sb")
            bg_sb = load(bgc, [P, 1], F32, "bg_sb")
            g2_sb = load(g2b, [P, D], F32, "g2_sb")
            bt2_sb = load(bt2b, [P, D], F32, "bt2_sb")
            idn_sb = load(idn, [P, P], BF, "idn_sb")
            ix_sb = [
                load(ixr[r], [P, max(NTr[r], 1) * 8], I16, f"ix{r}_sb")
                for r in range(NREG)
            ]
            eps_sb = sg.tile([P, 1], F32, name="eps_sb")
            nc.vector.memset(eps_sb[:], 1e-5)

            hT_sb = sg.tile([P, NLOC], BF, name="hT_sb")
            B_sb = sg.tile([P, NLOC], BF, name="B_sb")
            RW = max(REG_B[r + 1] - REG_B[r] for r in range(NREG)) * P
            hreg_sb = sg.tile([P, RW], BF, name="hreg_sb")
            areg_sb = sg.tile([P, RW], BF, name="areg_sb")

            HA_shard = dram.tile([NLOC, 2 * D], BF, name="HA_shard")
            HA_tab = [
                dram.tile(
                    [N_CORES * ROWS_R[r], 2 * D], BF, name=f"HA_t{r}",
                    addr_space="Shared",
                )
                for r in range(NREG)
            ]

            # ======== phase 1 + edges (one scope; gathers interleave) ========
            with (
                tc.tile_pool(name="pz", bufs=2, space="PSUM") as pz,
                tc.tile_pool(name="psc", bufs=2, space="PSUM") as psc,
                tc.tile_pool(name="pag", bufs=2, space="PSUM") as pag,
                tc.tile_pool(name="pg3", bufs=1, space="PSUM") as pg3,
                tc.tile_pool(name="gio", bufs=PF + 1) as gio,
                tc.tile_pool(name="gmk", bufs=PF + 1) as gmk,
                tc.tile_pool(name="wrk", bufs=3) as wrk,
                tc.tile_pool(name="osb", bufs=2) as osb,
            ):
                qctr = [0]

                def next_q():
                    q = qctr[0] % 4
                    qctr[0] += 1
                    return q

                # per-chunk buffer bookkeeping
                REG_OFF = [[0] for _ in range(NREG)]
                TOFF = [0]
                for (a, b) in chunks:
                    for r in range(NREG):
                        REG_OFF[r].append(REG_OFF[r][-1] + sum(TR[r][a:b]))
                    TOFF.append(TOFF[-1] + sum(T_all[a:b]))
                cbufs = {}

                def alloc_chunk(ci):
                    hab = [
                        gio.tile(
                            [P, max(TCr[r], 1), 2 * D], BF, tag=f"ha{r}",
                            name=f"ha{r}_{ci}",
                        )
                        for r in range(NREG)
                    ]
                    mkb = gmk.tile(
                        [P, max(TC, 1), 2 * P], BF, tag="mk", name=f"mk{ci}",
                    )
                    cbufs[ci] = (hab, mkb)
                    return hab, mkb

                def issue_mask(ci):
                    ntc = TOFF[ci + 1] - TOFF[ci]
                    mkb = cbufs[ci][1]
                    nc.sync.dma_start(
                        out=mkb[:, 0:ntc, :],
                        in_=masks[
                            :, TOFF[ci] * 2 * P : (TOFF[ci] + ntc) * 2 * P
                        ].rearrange("p (t c) -> p t c", t=ntc),
                    )

                def issue_gather(ci, r):
                    n = REG_OFF[r][ci + 1] - REG_OFF[r][ci]
                    if n:
                        off = REG_OFF[r][ci]
                        nc.gpsimd.dma_gather(
                            out_ap=cbufs[ci][0][r][:, 0:n, :],
                            in_ap=HA_tab[r][:, :],
                            idxs_ap=ix_sb[r][:, off * 8 : (off + n) * 8],
                            num_idxs=n * P,
                            num_idxs_reg=n * P,
                            elem_size=2 * D,
                            single_packet=False,
                            queue_num=next_q(),
                        )

                with tc.tile_pool(name="w1p", bufs=3) as w1p:
                  ha_v = HA_shard.rearrange("(g p) c -> p g c", p=P)
                  for r4 in range(NREG):
                      for g in range(REG_B[r4], REG_B[r4 + 1]):
                          gsl = slice(g * P, (g + 1) * P)
                          xg = w1p.tile([P, D], BF, tag="xg", name=f"xg{g}")
                          nc.sync.dma_start(out=xg[:], in_=xT[:, gsl])
                          hp = pz.tile([P, D], F32, tag="z", name=f"hp{g}")
                          nc.tensor.matmul(
                              out=hp[:], lhsT=xg[:], rhs=W1_sb[:],
                              start=True, stop=False,
                          )
                          nc.tensor.matmul(
                              out=hp[:], lhsT=o1b_sb[:], rhs=b1_sb[:],
                              start=False, stop=True,
                          )
                          st = w1p.tile([P, 6], F32, tag="st", name=f"st{g}")
                          nc.vector.bn_stats(out=st[:], in_=hp[:])
                          mv = w1p.tile([P, 2], F32, tag="mv", name=f"mv{g}")
                          nc.vector.bn_aggr(out=mv[:], in_=st[:])
                          sd = w1p.tile([P, 1], F32, tag="sd", name=f"sd{g}")
                          nc.scalar.activation(
                              out=sd[:], in_=mv[:, 1:2],
                              func=mybir.ActivationFunctionType.Sqrt,
                              bias=eps_sb[:],
                          )
                          rstd = w1p.tile([P, 1], F32, tag="rstd", name=f"rs{g}")
                          nc.vector.reciprocal(out=rstd[:], in_=sd[:])
                          t1 = w1p.tile([P, D], F32, tag="t1", name=f"t1{g}")
                          nc.vector.tensor_scalar(
                              out=t1[:], in0=hp[:], scalar1=mv[:, 0:1],
                              scalar2=rstd[:], op0=mybir.AluOpType.subtract,
                              op1=mybir.AluOpType.mult,
                          )
                          u1 = w1p.tile([P, D], F32, tag="u1", name=f"u1{g}")
                          nc.vector.tensor_mul(out=u1[:], in0=t1[:], in1=g1_sb[:])
                          v1 = w1p.tile([P, D], F32, tag="v1", name=f"v1{g}")
                          nc.vector.tensor_add(out=v1[:], in0=u1[:], in1=bt1_sb[:])
                          gl = slice((g - REG_B[r4]) * P, (g - REG_B[r4] + 1) * P)
                          hgt = hreg_sb[:, gl]
                          nc.vector.tensor_scalar_max(
                              out=hgt, in0=v1[:], scalar1=0.0
                          )
                          htp = pag.tile([P, D], F32, tag="aggr", name=f"htp{g}")
                          nc.tensor.matmul(
                              out=htp[:], lhsT=hgt, rhs=idn_sb[:],
                              start=True, stop=True,
                          )
                          nc.any.tensor_copy(out=hT_sb[:, gsl], in_=htp[:])
                          ap_ = psc.tile([P, D], F32, tag="s", name=f"apz{g}")
                          nc.tensor.matmul(
                              out=ap_[:], lhsT=hT_sb[:, gsl], rhs=We1T_sb[:],
                              start=True, stop=False,
                          )
                          nc.tensor.matmul(
                              out=ap_[:], lhsT=o1b_sb[:], rhs=be1_sb[:],
                              start=False, stop=True,
                          )
                          nc.any.tensor_copy(out=areg_sb[:, gl], in_=ap_[:])
                          bp = psc.tile([P, D], F32, tag="s", name=f"bp{g}")
                          nc.tensor.matmul(
                              out=bp[:], lhsT=hT_sb[:, gsl], rhs=We1B_sb[:],
                              start=True, stop=True,
                          )
                          nc.any.tensor_copy(out=B_sb[:, gsl], in_=bp[:])
                      # stage the region, then all-gather it
                      ga4, gb4 = REG_B[r4], REG_B[r4 + 1]
                      r0, r1 = ga4 * P, gb4 * P
                      ng4 = gb4 - ga4
                      nc.sync.dma_start(
                          out=ha_v[:, ga4:gb4, 0:D],
                          in_=hreg_sb[:, 0 : ng4 * P].rearrange(
                              "p (g j) -> p g j", g=ng4
                          ),
                      )
                      nc.sync.dma_start(
                          out=ha_v[:, ga4:gb4, D : 2 * D],
                          in_=areg_sb[:, 0 : ng4 * P].rearrange(
                              "p (g j) -> p g j", g=ng4
                          ),
                      )
                      nc.gpsimd.collective_compute(
                          "AllGather",
                          mybir.AluOpType.bypass,
                          replica_groups=[list(range(N_CORES))],
                          ins=[HA_shard[r0:r1, :].opt()],
                          outs=[HA_tab[r4][:, :].opt()],
                      )
                      # stagger pre-issued gathers so the next collective's
                      # dispatch is never head-blocked on the GpSimd queue
                      if r4 == 0:
                          for ci in range(min(PF, len(chunks))):
                              alloc_chunk(ci)
                              issue_mask(ci)
                      for ci in range(min(PF, len(chunks))):
                          issue_gather(ci, r4)

                def emit_tail(g, raggt):
                    gsl = slice(g * P, (g + 1) * P)
                    gp = pg3.tile([P, P], F32, tag="gp", name=f"gp{g}")
                    nc.tensor.matmul(
                        out=gp[:], lhsT=WgT_sb[:], rhs=hT_sb[:, gsl],
                        start=True, stop=False,
                    )
                    nc.tensor.matmul(
                        out=gp[:], lhsT=WgB_sb[:], rhs=raggt[:],
                        start=False, stop=True,
                    )
                    gate = wrk.tile([P, P], BF, tag="gate", name=f"gt_{g}")
                    nc.scalar.activation(
                        out=gate[:], in_=gp[:],
                        func=mybir.ActivationFunctionType.Sigmoid,
                        bias=bg_sb[:],
                    )
                    d1 = wrk.tile([P, P], BF, tag="d1", name=f"d1{g}")
                    nc.vector.tensor_tensor(
                        out=d1[:], in0=raggt[:], in1=hT_sb[:, gsl],
                        op=mybir.AluOpType.subtract,
                    )
                    d2 = wrk.tile([P, P], BF, tag="d2", name=f"d2{g}")
                    nc.vector.tensor_mul(out=d2[:], in0=gate[:], in1=d1[:])
                    hn = wrk.tile([P, P], BF, tag="hn", name=f"hn{g}")
                    nc.vector.tensor_add(
                        out=hn[:], in0=hT_sb[:, gsl], in1=d2[:]
                    )
                    hnp = pg3.tile([P, P], F32, tag="hnp", name=f"hnp{g}")
                    nc.tensor.matmul(
                        out=hnp[:], lhsT=hn[:], rhs=idn_sb[:],
                        start=True, stop=True,
                    )
                    st3 = wrk.tile([P, 6], F32, tag="st3", name=f"st3{g}")
                    nc.vector.bn_stats(out=st3[:], in_=hnp[:])
                    mv3 = wrk.tile([P, 2], F32, tag="mv3", name=f"mv3{g}")
                    nc.vector.bn_aggr(out=mv3[:], in_=st3[:])
                    sd3 = wrk.tile([P, 1], F32, tag="sd3", name=f"sd3{g}")
                    nc.scalar.activation(
                        out=sd3[:], in_=mv3[:, 1:2],
                        func=mybir.ActivationFunctionType.Sqrt,
                        bias=eps_sb[:],
                    )
                    rstd3 = wrk.tile([P, 1], F32, tag="rst3", name=f"rt{g}")
                    nc.vector.reciprocal(out=rstd3[:], in_=sd3[:])
                    t1o = osb.tile([P, D], F32, tag="t1o", name=f"t1o{g}")
                    nc.vector.tensor_scalar(
                        out=t1o[:], in0=hnp[:],
                        scalar1=mv3[:, 0:1], scalar2=rstd3[:],
                        op0=mybir.AluOpType.subtract,
                        op1=mybir.AluOpType.mult,
                    )
                    u1o = osb.tile([P, D], F32, tag="u1o", name=f"u1o{g}")
                    nc.vector.tensor_mul(out=u1o[:], in0=t1o[:], in1=g2_sb[:])
                    o1o = osb.tile([P, D], F32, tag="o1o", name=f"o1o{g}")
                    nc.vector.tensor_add(out=o1o[:], in0=u1o[:], in1=bt2_sb[:])
                    nc.sync.dma_start(out=out[gsl, :], in_=o1o[:])

                prev = None
                for ci, (ga, gb) in enumerate(chunks):
                    if ci + PF < len(chunks):
                        alloc_chunk(ci + PF)
                        issue_mask(ci + PF)
                        for r in range(NREG):
                            issue_gather(ci + PF, r)
                    habuf, mkbuf = cbufs.pop(ci)
                    for g in range(ga, gb):
                        tr = [TR[r][g] for r in range(NREG)]
                        Tg = sum(tr)
                        gsl = slice(g * P, (g + 1) * P)
                        _rp = [
                            sum(TR[r][ga:g]) for r in range(NREG)
                        ]
                        _m = sum(T_all[ga:g])
                        c0 = tr[0]

                        def hat(j, _rp=_rp, _c0=c0):
                            if j < _c0:
                                return habuf[0][:, _rp[0] + j, :]
                            return habuf[1][:, _rp[1] + j - _c0, :]

                        s_ps = psc.tile([P, Tg], F32, tag="s", name=f"s{g}")
                        w_sb = wrk.tile([P, Tg], BF, tag="w", name=f"w{g}")
                        nck = (Tg + SB - 1) // SB
                        for c in range(nck):
                            tl = c * SB
                            th = min(tl + SB, Tg)
                            wl = (th - tl) * P
                            z = pz.tile(
                                [P, SB * P], F32, tag="z", name=f"z{g}_{c}"
                            )
                            # start=True resets the whole PSUM bank's
                            # has_written bits: full-region write first,
                            # per-slice accumulates after
                            nc.tensor.matmul(
                                out=z[:, 0:wl], lhsT=B_sb[:, gsl],
                                rhs=mkbuf[:, _m + tl : _m + th, 0:P],
                                start=True, stop=False,
                            )
                            for i, t in enumerate(range(tl, th)):
                                zsl = slice(i * P, (i + 1) * P)
                                nc.tensor.matmul(
                                    out=z[:, zsl], lhsT=hat(t)[:, D : 2 * D],
                                    rhs=idn_sb[:], start=False,
                                    stop=(t == th - 1),
                                )
                            r_ = wrk.tile(
                                [P, SB * P], BF, tag="r", name=f"r{g}_{c}"
                            )
                            nc.vector.tensor_scalar_max(
                                out=r_[:, 0:wl], in0=z[:, 0:wl], scalar1=0.0
                            )
                            for i, t in enumerate(range(tl, th)):
                                nc.tensor.matmul(
                                    out=s_ps[:, t : t + 1],
                                    lhsT=r_[:, i * P : (i + 1) * P],
                                    rhs=We2_sb[:], start=True, stop=True,
                                )
                        nc.scalar.activation(
                            out=w_sb[:], in_=s_ps[:, 0:Tg],
                            func=mybir.ActivationFunctionType.Sigmoid,
                            bias=be2_sb[:],
                        )
                        # deferred tail of the previous group first, so
                        # its Vector ops aren't stuck behind wh's sem wait
                        if prev is not None:
                            emit_tail(*prev)
                        # weighted source tiles: broadcast tensor_tensor
                        # multiply (w stride-0 along D) off the gather bufs
                        wh = wrk.tile([P, Tg, D], BF, tag="hc", name=f"wh{g}")
                        woff0 = 0
                        for r in range(NREG):
                            if tr[r]:
                                nc.vector.tensor_tensor(
                                    out=wh[:, woff0 : woff0 + tr[r], :],
                                    in0=habuf[r][
                                        :, _rp[r] : _rp[r] + tr[r], 0:D
                                    ],
                                    in1=_bcast_d(w_sb[:, woff0 : woff0 + tr[r]]),
                                    op=mybir.AluOpType.mult,
                                )
                                woff0 += tr[r]
                        aggr = pag.tile([P, P], F32, tag="aggr", name=f"ag{g}")
                        for t in range(Tg):
                            nc.tensor.matmul(
                                out=aggr[:], lhsT=wh[:, t, :],
                                rhs=mkbuf[:, _m + t, P : 2 * P],
                                start=(t == 0), stop=(t == Tg - 1),
                            )
                        raggt = wrk.tile([P, P], BF, tag="ragg", name=f"rg{g}")
                        nc.vector.tensor_copy(out=raggt[:], in_=aggr[:])
                        prev = (g, raggt)
                emit_tail(*prev)

    nc.compile()
    return nc


# ----------------------------------------------------------------------------
# host-side sharding + launch
# ----------------------------------------------------------------------------

_CACHE = {}


def _wrap16(seq):
    """idx i -> [i%16, i//16], replicated to 128 partitions (8 Q7 cores)."""
    n = len(seq)
    if n == 0:
        return np.zeros((P, 8), np.int16)
    assert n % 16 == 0
    blk = np.asarray(seq, np.int16).reshape(-1, 16).T
    return np.tile(blk, (8, 1))


def kernel(
    x, edge_index, W1, b1, g1, bt1, We1, be1, We2, be2,
    Wn1, bn1, Wn2, bn2, Wg, bg, g2, bt2, _trace=False,
):
    x = np.asarray(x, dtype=np.float32)
    N = x.shape[0]
    NG = (N + N_CORES * P - 1) // (N_CORES * P)
    NLOC = NG * P
    NPAD = NLOC * N_CORES

    assert sum(REG_G) == NG
    REG_B = [0]
    for s in REG_G:
        REG_B.append(REG_B[-1] + s)
    ROWS_R = [REG_G[r] * P for r in range(NREG)]     # rows per core per region
    assert all(N_CORES * rr <= 32768 for rr in ROWS_R)

    row = np.asarray(edge_index[0], dtype=np.int64)
    col = np.asarray(edge_index[1], dtype=np.int64)
    order = np.argsort(col, kind="stable")
    row_s = row[order].astype(np.int64)
    col_s = col[order].astype(np.int64)
    bounds = np.searchsorted(col_s, np.arange(N_CORES + 1) * NLOC)

    # remap src node -> (region table, row within table)
    k_s = row_s // NLOC
    r_s = row_s % NLOC
    gr_s = r_s // P
    reg_s = np.searchsorted(np.asarray(REG_B[1:]), gr_s, side="right")
    rows_r_a = np.asarray(ROWS_R)
    reg_b_a = np.asarray(REG_B)
    tabrow = (k_s * rows_r_a[reg_s] + (r_s - reg_b_a[reg_s] * P)).astype(
        np.int64
    )

    # per-core per-group per-region counts -> shared tile counts
    cnt = np.zeros((NREG, N_CORES, NG), np.int64)
    for k in range(N_CORES):
        lo, hi = bounds[k], bounds[k + 1]
        gk = (col_s[lo:hi] - k * NLOC) // P
        rg = reg_s[lo:hi]
        for r in range(NREG):
            cnt[r, k] = np.bincount(gk[rg == r], minlength=NG)
    TR = [
        [int(v) for v in np.ceil(cnt[r].max(axis=0) / P)] for r in range(NREG)
    ]
    for g in range(NG):
        if sum(TR[r][g] for r in range(NREG)) == 0:
            TR[0][g] = 1
    T_all = [sum(TR[r][g] for r in range(NREG)) for g in range(NG)]
    NT = sum(T_all)
    toff = np.concatenate([[0], np.cumsum(T_all)]).astype(np.int64)

    key = (N, NG, tuple(tuple(TR[r]) for r in range(NREG)))
    if key not in _CACHE:
        _CACHE[key] = _build_program(NG, NLOC, NPAD, TR, REG_B)
    nc = _CACHE[key]

    bf = lambda a: np.ascontiguousarray(np.asarray(a, np.float32)).astype(BF16)
    f32 = lambda a: np.ascontiguousarray(np.asarray(a, np.float32))
    shared = {
        "W1": bf(W1),
        "b1row": bf(b1).reshape(1, D),
        "ones1b": np.ones((1, D), BF16),
        "g1b": np.broadcast_to(f32(g1).reshape(1, D), (P, D)).copy(),
        "bt1b": np.broadcast_to(f32(bt1).reshape(1, D), (P, D)).copy(),
        "We1T": bf(We1[:D]),
        "We1B": bf(We1[D:]),
        "be1row": bf(be1).reshape(1, D),
        "We2c": bf(We2).reshape(P, 1),
        "be2c": np.broadcast_to(f32(be2).reshape(1, 1), (P, 1)).copy(),
        "WgT": bf(Wg[:D]),
        "WgB": bf(Wg[D:]),
        "bgc": f32(bg).reshape(P, 1),
        "g2b": np.broadcast_to(f32(g2).reshape(1, D), (P, D)).copy(),
        "bt2b": np.broadcast_to(f32(bt2).reshape(1, D), (P, D)).copy(),
        "idn": np.eye(P, dtype=BF16),
    }

    xp = np.zeros((NPAD, D), np.float32)
    xp[:N] = x

    in_maps = []
    for k in range(N_CORES):
        lo, hi = bounds[k], bounds[k + 1]
        rk = tabrow[lo:hi]
        regk = reg_s[lo:hi]
        ck = col_s[lo:hi] - k * NLOC
        gk = ck // P
        crel = (ck % P).astype(np.int64)
        seqs = [[] for _ in range(NREG)]
        M = np.zeros((NT, P, 2 * P), BF16)
        one = BF16(1.0)
        for g in range(NG):
            gm = gk == g
            r_g = rk[gm]
            c_g = crel[gm]
            reg_g = regk[gm]
            woff = 0
            for r in range(NREG):
                sel = reg_g == r
                Tcnt = TR[r][g]
                rr = r_g[sel]
                cc = c_g[sel]
                o = np.argsort(rr, kind="stable")
                rr = rr[o]
                cc = cc[o]
                n = len(rr)
                idxs = np.zeros(Tcnt * P, np.int64)
                idxs[:n] = rr
                seqs[r].append(idxs)
                if n:
                    e = np.arange(n)
                    t_proc = toff[g] + woff + e // P
                    slot = e % P
                    M[t_proc, cc, slot] = one
                    M[t_proc, slot, P + cc] = one
                woff += Tcnt
        im = dict(shared)
        im["xT"] = np.ascontiguousarray(
            xp[k * NLOC : (k + 1) * NLOC].T
        ).astype(BF16)
        for r in range(NREG):
            s = (
                np.concatenate(seqs[r]) if seqs[r] else np.zeros(0, np.int64)
            )
            im[f"ix{r}"] = _wrap16(s)
        im["masks"] = np.ascontiguousarray(
            M.transpose(1, 0, 2).reshape(P, NT * 2 * P)
        )
        in_maps.append(im)

    if _trace:
        _install_ntff_hook()
    res = run_bass_kernel_spmd(
        nc, in_maps, core_ids=list(range(N_CORES)), trace=_trace
    )
    out = np.concatenate(
        [res.results[k]["out"] for k in range(N_CORES)], axis=0
    )[:N]
    if _trace:
        kernel.last_exec_time_ns = res.exec_time_ns
    return np.ascontiguousarray(out, dtype=np.float32)
